# revision 9
# baseline (speedup 1.0000x reference)
"""Trainium2 Bass kernel for nn_Decoder: 2-layer LSTM decoder + log-softmax NLL.

Fully transposed recurrence, fp8 DoubleRow matmuls.

Per core (8-way data parallel over batch, BL=32 rows/core, zero collectives):
- All matmuls keep WEIGHTS stationary ([K=128, M=128] full PE tiles) and
  stream transposed activations (N=32 columns); no transposes in the loop.
- fp8e4m3 + MatmulPerfMode.DoubleRow (2 k-tiles per pass, 0.5 cycles/row) for
  the e/h gate matmuls and the vocab projection. zpre/bias/zlog injects bf16.
- Gates: sigmoid(x) = 0.5 + 0.5*tanh(x/2) with input scales folded into
  host-prescaled weights -> one tanh ACT per layer; tanh+exp share the single
  `exp_and_others` ACT table (2 table loads total).
- Cell state kept doubled (s = 2c); gate math = 3 scalar_tensor_tensor ops +
  1 tanh + 1 STT per layer (exact algebra; fixups folded into weights).
- transformh0 / zpre / zlog / target-z-part computed on host (z-only, tiny).
- PSUM accumulation: ONE start/stop pair per PSUM tile (slice-level start
  flags re-arm the zero region and wipe earlier slices on hardware).
"""

import numpy as np
import ml_dtypes

import concourse.tile as tile
import concourse.mybir as mybir
from concourse import bacc
from concourse import bass_utils

B, T, V, D, Z = 256, 40, 5000, 512, 128
NC = 8
BL = B // NC              # 32 batch rows per core
NT = T - 1                # 39 recurrent steps
COLS = NT * BL            # 1248 (t, b) columns per core
G = 4 * D                 # 2048 gate width
NTILE = (COLS + 127) // 128   # 10 vocab tiles (last has 96 cols)
NVG = (V + 1023) // 1024      # 5 vocab exp groups per tile (last 904)

bf16 = mybir.dt.bfloat16
f8 = mybir.dt.float8e4
f32 = mybir.dt.float32
AF = mybir.ActivationFunctionType
ALU = mybir.AluOpType
DR = mybir.MatmulPerfMode.DoubleRow

VOC_START = 6      # first step allowed to pump vocab work
VOC_PACE = 2       # vocab 1024-groups pumped per step
DEBUG = False

_CACHE = {}


def _build():
    nc = bacc.Bacc("TRN2", target_bir_lowering=False, debug=False)

    def din(name, shape, dt):
        return nc.dram_tensor(name, shape, dt, kind="ExternalInput").ap()

    id32_d = din("id32", [32, 32], bf16)
    ones32_d = din("ones32", [1, 32], bf16)
    onescol_d = din("onescol", [128, 2], bf16)
    selb_d = din("selb", [32, 128], bf16)
    zpre_d = din("zpre", [32, G], bf16)
    zlog_d = din("zlog", [32, V], bf16)
    bg1_d = din("bg1r", [1, G], bf16)
    h0i_d = din("h0i", [128, 128], f8)
    h1i_d = din("h1i", [128, 128], f8)
    s0i_d = din("s0i", [128, 128], f32)
    s1i_d = din("s1i", [128, 128], f32)
    w0e_d = din("w0e", [128, 4 * G], f8)
    eT_d = din("eT", [128, 4 * NT * BL], f8)
    w0h_d = din("w0h", [128, 4 * G], f8)
    w1_d = din("w1", [128, 8 * G], f8)
    wout_d = din("wout", [128, 4 * V], f8)
    wta_d = din("wta", [128, 4 * COLS], f8)
    out_d = nc.dram_tensor("out_lp", [COLS, 1], f32, kind="ExternalOutput").ap()
    if DEBUG:
        dbg_ht_d = nc.dram_tensor("dbg_ht", [128, 4 * COLS], bf16,
                                  kind="ExternalOutput").ap()
        dbg_sums_d = nc.dram_tensor("dbg_sums", [128, 16], f32,
                                    kind="ExternalOutput").ap()
        dbg_dps_d = nc.dram_tensor("dbg_dps", [128, 16], f32,
                                   kind="ExternalOutput").ap()
        dbg_ta0_d = nc.dram_tensor("dbg_ta0", [128, 512], bf16,
                                   kind="ExternalOutput").ap()
        dbg_ta1_d = nc.dram_tensor("dbg_ta1", [128, 512], bf16,
                                   kind="ExternalOutput").ap()
        dbg_h0s_d = nc.dram_tensor("dbg_h0s", [128, 128], bf16,
                                   kind="ExternalOutput").ap()
        dbg_g0_d = nc.dram_tensor("dbg_g0", [128, 512], f32,
                                  kind="ExternalOutput").ap()

    with tile.TileContext(nc) as tc:
        from contextlib import ExitStack
        with ExitStack() as ctx:
            const = ctx.enter_context(tc.tile_pool(name="const", bufs=1))

            def cload(shape, dt, dram, tag):
                t = const.tile(shape, dt, tag=tag)
                nc.sync.dma_start(t[:], dram[:])
                return t

            id32 = cload([32, 32], bf16, id32_d, "id32")
            ones32 = cload([1, 32], bf16, ones32_d, "ones32")
            onescol = cload([128, 2], bf16, onescol_d, "onescol")
            selb = cload([32, 128], bf16, selb_d, "selb")
            zpre = cload([32, G], bf16, zpre_d, "zpre")
            bg1 = cload([1, G], bf16, bg1_d, "bg1")
            h0i = cload([128, 4, 32], f8, h0i_d, "h0i")
            h1i = cload([128, 4, 32], f8, h1i_d, "h1i")
            s0i = cload([128, 128], f32, s0i_d, "s0i")
            s1i = cload([128, 128], f32, s1i_d, "s1i")
            zlog = cload([32, V], bf16, zlog_d, "zlog")

            wpool = ctx.enter_context(tc.tile_pool(name="w", bufs=1))
            # DoubleRow layouts: [q, pair g, ktile p, ...]
            w0e = wpool.tile([128, 2, 2, G], f8)
            nc.sync.dma_start(w0e[:], w0e_d[:])
            eT = wpool.tile([128, 2, 2, NT * BL], f8)
            nc.sync.dma_start(eT[:], eT_d[:])
            w0h = wpool.tile([128, 2, 2, G], f8)
            nc.sync.dma_start(w0h[:], w0h_d[:])
            w1 = wpool.tile([128, 4, 2, G], f8)
            nc.sync.dma_start(w1[:], w1_d[:])
            wout = wpool.tile([128, 2, 2, V], f8)
            nc.sync.dma_start(wout[:], wout_d[:])
            wta = wpool.tile([128, 4, COLS], f8)
            nc.sync.dma_start(wta[:], wta_d[:])

            state = ctx.enter_context(tc.tile_pool(name="state", bufs=1))
            HT = state.tile([128, 4, COLS], f8)
            sums_all = state.tile([128, 16], f32, tag="sums_all")
            nc.vector.memset(sums_all[:], 1.0)
            dps_all = state.tile([128, 16], f32, tag="dps_all")
            nc.vector.memset(dps_all[:], 0.0)

            sact = ctx.enter_context(tc.tile_pool(name="sact", bufs=2))
            sdve = ctx.enter_context(tc.tile_pool(name="sdve", bufs=2))
            sst = ctx.enter_context(tc.tile_pool(name="sst", bufs=2))
            sexp = ctx.enter_context(tc.tile_pool(name="sexp", bufs=2))
            gsum = ctx.enter_context(tc.tile_pool(name="gsum", bufs=2))
            pvoc_cm = tc.tile_pool(name="pvoc", bufs=2, space="PSUM")
            pvoc = pvoc_cm.__enter__()

            # ---------------- vocab pump machinery -------------------------
            gsums = {}
            vwork = []
            vpushed = 0

            def emit_vgroup(j, vi):
                base = 128 * j
                mj = min(128, COLS - base)
                vg0 = 1024 * vi
                vgs = min(1024, V - vg0)
                pl = pvoc.tile([128, 1024], f32, tag="pl")
                # DR moving operand free size is 2*vs -> keep vs <= 256.
                # pl spans TWO psum banks: the zero region is per-bank, so the
                # first matmul touching EACH bank must carry start=True and
                # the last one stop=True.
                for q in range(0, vgs, 256):
                    v0 = vg0 + q
                    vs = min(256, V - v0)
                    qs = slice(q, q + vs)
                    for g in range(2):
                        nc.tensor.matmul(
                            pl[:mj, qs], HT[:, 2 * g:2 * g + 2, base:base + mj],
                            wout[:, g, :, v0:v0 + vs],
                            start=(q % 512 == 0 and g == 0), stop=False,
                            perf_mode=DR, skip_group_check=True)
                for half in range(0, vgs, 512):
                    v0 = vg0 + half
                    vs = min(512, V - v0)
                    nc.tensor.matmul(pl[:mj, half:half + vs], selb[:, 0:mj],
                                     zlog[:, v0:v0 + vs],
                                     start=False, stop=True,
                                     skip_group_check=True)
                es = sexp.tile([128, 1024], bf16, tag="es")
                if vi == 0:
                    gsums[j] = gsum.tile([128, 8], f32, tag="gs",
                                         name=f"gs{j}")
                nc.scalar.activation(es[:mj, 0:vgs], pl[:mj, 0:vgs], AF.Exp,
                                     accum_out=gsums[j][:mj, vi:vi + 1])
                if vi == NVG - 1:
                    nc.vector.tensor_reduce(
                        sums_all[:mj, j:j + 1], gsums[j][:mj, 0:NVG],
                        mybir.AxisListType.XYZW, ALU.add)

            def vocab_pump(t_done, n):
                nonlocal vpushed
                while vpushed < NTILE and min(4 * vpushed + 4, NT - 1) <= t_done:
                    for vi in range(NVG):
                        vwork.append((vpushed, vi))
                    vpushed += 1
                for _ in range(n):
                    if not vwork:
                        return
                    j, vi = vwork.pop(0)
                    emit_vgroup(j, vi)

            # ---------------- main recurrent loop --------------------------
            with tc.tile_pool(name="p0g", bufs=2, space="PSUM") as p0g, \
                 tc.tile_pool(name="p1g", bufs=2, space="PSUM") as p1g:

                h0s_of = {-1: h0i}
                h1s_of = {-2: h1i, -1: h1i}
                s0_prev = s0i
                s1_prev = s1i
                g1_of = {}

                def tail(layer, t, gp, s_prev, HT_write):
                    tA = sact.tile([128, 512], bf16, tag=f"a{layer}",
                                   name=f"tA{layer}")
                    if DEBUG and t == 0 and layer == 0:
                        gcopy = sact.tile([128, 512], f32, tag="gcopy")
                        nc.vector.tensor_copy(gcopy[:], gp[:])
                        nc.sync.dma_start(dbg_g0_d[:], gcopy[:])
                    nc.scalar.activation(tA[:], gp[:], AF.Tanh)
                    if DEBUG and t == 0 and layer == 0:
                        nc.sync.dma_start(dbg_ta0_d[:], tA[:])
                    if DEBUG and t == 0 and layer == 1:
                        nc.sync.dma_start(dbg_ta1_d[:], tA[:])
                    u1 = sdve.tile([128, 128], f32, tag=f"u1{layer}",
                                   name=f"u1{layer}")
                    nc.vector.scalar_tensor_tensor(
                        u1[:], tA[:, 0:128], 1.0, s_prev[:], ALU.add, ALU.mult)
                    u2 = sdve.tile([128, 128], f32, tag=f"u2{layer}",
                                   name=f"u2{layer}")
                    nc.vector.scalar_tensor_tensor(
                        u2[:], tA[:, 128:256], 1.0, tA[:, 256:384],
                        ALU.add, ALU.mult)
                    sn = sst.tile([128, 128], f32, tag=f"s{layer}",
                                  name=f"sn{layer}")
                    nc.vector.scalar_tensor_tensor(
                        sn[:], u1[:], 0.5, u2[:], ALU.mult, ALU.add)
                    th = sdve.tile([128, 128], bf16, tag=f"th{layer}",
                                   name=f"th{layer}")
                    nc.scalar.activation(th[:], sn[:], AF.Tanh, scale=0.5)
                    hsn = sst.tile([128, 4, 32], f8, tag=f"h{layer}s",
                                   name=f"hsn{layer}")
                    nc.vector.scalar_tensor_tensor(
                        hsn[:], tA[:, 384:512], 1.0, th[:], ALU.add, ALU.mult)
                    if DEBUG and t == 0 and layer == 0:
                        h8c = sact.tile([128, 128], bf16, tag="h8c")
                        nc.vector.tensor_copy(h8c[:], hsn[:])
                        nc.sync.dma_start(dbg_h0s_d[:], h8c[:])
                    if HT_write is not None:
                        tw = HT_write
                        for c in range(4):
                            nc.vector.tensor_add(
                                HT[:, c, 32 * tw:32 * tw + 32],
                                h0s_of[tw][:, c, :], hsn[:, c, :])
                    return hsn, sn

                for t in range(NT):
                    # l0 gate group: e-part + zpre (no recurrence deps)
                    g0 = p0g.tile([128, 512], f32, tag="g0")
                    for m in range(16):
                        ms = slice(32 * m, 32 * m + 32)
                        js = slice(128 * m, 128 * m + 128)
                        for g in range(2):
                            nc.tensor.matmul(
                                g0[:, ms], w0e[:, g, :, js],
                                eT[:, g, :, BL * t:BL * t + BL],
                                start=(m == 0 and g == 0), stop=False,
                                perf_mode=DR, skip_group_check=True)
                        nc.tensor.matmul(g0[:, ms], zpre[:, js],
                                         id32[:, 0:32], start=False, stop=False,
                                         skip_group_check=True)

                    # l1(t-1) part A: h1(t-2) chunks + bias
                    if t > 0:
                        g1 = p1g.tile([128, 512], f32, tag="g1")
                        g1_of[t - 1] = g1
                        h1p = h1s_of[t - 2]
                        for m in range(16):
                            ms = slice(32 * m, 32 * m + 32)
                            js = slice(128 * m, 128 * m + 128)
                            for g in range(2):
                                nc.tensor.matmul(
                                    g1[:, ms], w1[:, g, :, js],
                                    h1p[:, 2 * g:2 * g + 2, :],
                                    start=(m == 0 and g == 0), stop=False,
                                    perf_mode=DR, skip_group_check=True)
                            nc.tensor.matmul(g1[:, ms], bg1[0:1, js],
                                             ones32[0:1, 0:32],
                                             start=False, stop=False,
                                             skip_group_check=True)

                    # vocab PE+ACT filler
                    if t >= VOC_START:
                        vocab_pump(t - 2, VOC_PACE)

                    # l0 h-part (closes g0) -- needs h0s(t-1)
                    h0p = h0s_of[t - 1]
                    for m in range(16):
                        ms = slice(32 * m, 32 * m + 32)
                        js = slice(128 * m, 128 * m + 128)
                        for g in range(2):
                            nc.tensor.matmul(
                                g0[:, ms], w0h[:, g, :, js],
                                h0p[:, 2 * g:2 * g + 2, :],
                                start=False,
                                stop=(m == 15 and g == 1),
                                perf_mode=DR, skip_group_check=True)

                    # l1(t-1) part B: h0(t-1) chunks (closes g1)
                    if t > 0:
                        g1 = g1_of[t - 1]
                        for m in range(16):
                            ms = slice(32 * m, 32 * m + 32)
                            js = slice(128 * m, 128 * m + 128)
                            for g in range(2):
                                nc.tensor.matmul(
                                    g1[:, ms], w1[:, 2 + g, :, js],
                                    h0p[:, 2 * g:2 * g + 2, :],
                                    start=False,
                                    stop=(m == 15 and g == 1),
                                    perf_mode=DR, skip_group_check=True)

                    # elementwise tails
                    h0s, s0n = tail(0, t, g0, s0_prev, None)
                    h0s_of[t] = h0s
                    s0_prev = s0n
                    if t > 0:
                        h1s, s1n = tail(1, t - 1, g1_of.pop(t - 1), s1_prev,
                                        t - 1)
                        h1s_of[t - 1] = h1s
                        s1_prev = s1n
                        del h1s_of[t - 3]
                        del h0s_of[t - 2]

                # flush l1(NT-1)
                t = NT
                g1 = p1g.tile([128, 512], f32, tag="g1")
                h1p = h1s_of[t - 2]
                h0p = h0s_of[t - 1]
                for m in range(16):
                    ms = slice(32 * m, 32 * m + 32)
                    js = slice(128 * m, 128 * m + 128)
                    for g in range(2):
                        nc.tensor.matmul(g1[:, ms], w1[:, g, :, js],
                                         h1p[:, 2 * g:2 * g + 2, :],
                                         start=(m == 0 and g == 0), stop=False,
                                         perf_mode=DR, skip_group_check=True)
                    nc.tensor.matmul(g1[:, ms], bg1[0:1, js],
                                     ones32[0:1, 0:32], start=False, stop=False,
                                     skip_group_check=True)
                    for g in range(2):
                        nc.tensor.matmul(g1[:, ms], w1[:, 2 + g, :, js],
                                         h0p[:, 2 * g:2 * g + 2, :],
                                         start=False,
                                         stop=(m == 15 and g == 1),
                                         perf_mode=DR, skip_group_check=True)
                tail(1, NT - 1, g1, s1_prev, NT - 1)

            # remaining vocab + target-dot + finalize
            with tc.tile_pool(name="pdot", bufs=2, space="PSUM") as pdot, \
                 tc.tile_pool(name="ptt", bufs=2) as ptt:
                vocab_pump(NT - 1, 2)
                for j in range(NTILE):
                    base = 128 * j
                    mj = min(128, COLS - base)
                    vocab_pump(NT - 1, NVG)
                    dps = pdot.tile([128, 2], f32, tag="dps")
                    for c in range(4):
                        sc = ptt.tile([128, 128], bf16, tag="sc")
                        nc.vector.tensor_mul(sc[:, 0:mj],
                                             HT[:, c, base:base + mj],
                                             wta[:, c, base:base + mj])
                        nc.tensor.matmul(dps[:mj, 0:2], sc[:, 0:mj],
                                         onescol[:, 0:2],
                                         start=(c == 0), stop=(c == 3))
                    nc.vector.tensor_copy(dps_all[:mj, j:j + 1], dps[:mj, 0:1])
                vocab_pump(NT - 1, len(vwork) + 4)

                lses = ptt.tile([128, 16], f32, tag="lses")
                nc.scalar.activation(lses[:], sums_all[:], AF.Ln)
                lpd = ptt.tile([128, 16], f32, tag="lpd")
                nc.vector.tensor_sub(lpd[:], dps_all[:], lses[:])
                for j in range(NTILE):
                    base = 128 * j
                    mj = min(128, COLS - base)
                    nc.sync.dma_start(out_d[base:base + mj, 0:1],
                                      lpd[:mj, j:j + 1])
                if DEBUG:
                    htc = state.tile([128, 4 * COLS], bf16, tag="htc")
                    nc.vector.tensor_copy(htc[:], HT[:])
                    nc.sync.dma_start(dbg_ht_d[:], htc[:])
                    nc.sync.dma_start(dbg_sums_d[:], sums_all[:])
                    nc.sync.dma_start(dbg_dps_d[:], dps_all[:])
            pvoc_cm.__exit__(None, None, None)

    nc.compile()
    return nc


def _prep_host(inputs):
    z = np.asarray(inputs["z"], np.float32)
    x = np.asarray(inputs["x"])
    emb = np.asarray(inputs["emb"], np.float32)
    Wg0 = np.asarray(inputs["Wg0"], np.float32)
    bg0 = np.asarray(inputs["bg0"], np.float32)
    Wg1 = np.asarray(inputs["Wg1"], np.float32)
    bg1 = np.asarray(inputs["bg1"], np.float32)
    Wout = np.asarray(inputs["Wout"], np.float32)
    bout = np.asarray(inputs["bout"], np.float32)
    tw1 = np.asarray(inputs["tw1"], np.float32)
    tb1 = np.asarray(inputs["tb1"], np.float32)
    tw2 = np.asarray(inputs["tw2"], np.float32)
    tb2 = np.asarray(inputs["tb2"], np.float32)

    bf = ml_dtypes.bfloat16
    f8h = ml_dtypes.float8_e4m3fn

    def permute_rows(W):
        # (i, f, o, cn) blocks -> (f, i, cn, o)
        return np.concatenate(
            [W[512:1024], W[0:512], W[1536:2048], W[1024:1536]], axis=0)

    rs = np.repeat([0.5, 0.5, 1.0, 0.5], 512).astype(np.float32)[:, None]

    def chunked(a, nch):
        # [128*nch, N] -> [128, nch*N] (chunk-major free layout)
        n = a.shape[1]
        return np.ascontiguousarray(
            a.reshape(nch, 128, n).transpose(1, 0, 2).reshape(128, nch * n))

    W0 = Wg0.reshape(G, D + Z + D)
    W0p = permute_rows(W0) * rs
    bg0p = (permute_rows(bg0.reshape(G, 1)) * rs)[:, 0]
    W1 = Wg1.reshape(G, 2 * D)
    W1p = permute_rows(W1) * rs * 0.5
    bg1p = (permute_rows(bg1.reshape(G, 1)) * rs)[:, 0]
    W0z_s = W0p[:, 1024:1152]

    shared = {
        # chunked() already yields the [q, chunk, ...] layout; DoubleRow just
        # reinterprets chunk index as (pair g, ktile p)
        "w0h": chunked(W0p[:, 0:512].T * 0.5, 4).astype(f8h),
        "w0e": chunked(W0p[:, 512:1024].T, 4).astype(f8h),
        "w1": chunked(W1p.T, 8).astype(f8h),
        "bg1r": bg1p.reshape(1, G).astype(bf),
        "wout": chunked(Wout[:, 0:512].T * 0.5, 4).astype(f8h),
        "id32": np.eye(32, dtype=bf),
        "ones32": np.ones((1, 32), bf),
        "onescol": np.ones((128, 2), bf),
        "selb": np.tile(np.eye(32, dtype=bf), (1, 4)),
    }

    def packT(a):  # [32, 512] -> [128, 128] transposed chunk-packed
        return np.ascontiguousarray(
            a.T.reshape(4, 128, 32).transpose(1, 0, 2).reshape(128, 128))

    in_maps = []
    extra = []
    for cidx in range(NC):
        bs = slice(BL * cidx, BL * cidx + BL)
        z_c = z[bs]
        x_c = x[bs]
        xn = x_c[:, 1:T]

        m = dict(shared)
        for l in range(2):
            u = np.maximum(z_c @ tw1[l].T + tb1[l], 0.0)
            hh = np.tanh(u @ tw2[l].T + tb2[l])
            m[f"h{l}i"] = packT(2.0 * hh[:, 0:512]).astype(f8h)
            m[f"s{l}i"] = packT(2.0 * hh[:, 512:1024]).astype(np.float32)

        m["zpre"] = (z_c @ W0z_s.T + bg0p).astype(bf)
        zlog_f = z_c @ Wout[:, 512:640].T + bout
        m["zlog"] = zlog_f.astype(bf)
        tdz = np.take_along_axis(zlog_f, xn, axis=1)
        extra.append(tdz.sum(axis=1))

        embx = emb[x_c[:, 0:NT]]
        m["eT"] = np.ascontiguousarray(
            embx.transpose(2, 1, 0).reshape(4, 128, NT * BL)
            .transpose(1, 0, 2).reshape(128, 4 * NT * BL)).astype(f8h)
        wrows = Wout[xn][:, :, 0:512] * 0.5
        m["wta"] = np.ascontiguousarray(
            wrows.transpose(2, 1, 0).reshape(4, 128, COLS)
            .transpose(1, 0, 2).reshape(128, 4 * COLS)).astype(f8h)
        in_maps.append(m)
    return in_maps, extra


def kernel(**inputs) -> np.ndarray:
    if "nc" not in _CACHE:
        _CACHE["nc"] = _build()
    nc = _CACHE["nc"]
    in_maps, extra = _prep_host(inputs)
    res = bass_utils.run_bass_kernel_spmd(nc, in_maps, core_ids=list(range(NC)))
    out = np.zeros((B, 1), np.float32)
    for cidx in range(NC):
        lp = res.results[cidx]["out_lp"].reshape(NT, BL)
        out[BL * cidx:BL * cidx + BL, 0] = lp.sum(axis=0) + extra[cidx]
    return out
